# revision 13
# baseline (speedup 1.0000x reference)
"""BlurPool3D Trainium2 kernel.

Depthwise 5x5x5 binomial blur, stride 2, edge padding, on x [2, 32, 96, 96, 96] f32.
Output [2, 32, 48, 48, 48] f32.

Strategy (per NeuronCore, channels sharded 8-ways -> 8 (b,c) planes/core):
  The filter is separable: f = f1 x f1 x f1, f1 = [1,4,6,4,1]/16.
  Per plane [D, H, W] = [96, 96, 96], no transposes anywhere:

  S1 (TensorE: contract D + shift-accumulate H): SBUF X [96(d), 100(h pad), 96(w)].
     For each h'-block, 5 PSUM-accumulated matmuls (one per H tap kh):
       psum[d', (h',w)] += (f1[kh] * A_D^T).T @ X[:, 2h'+kh, :]
     where A_D [48,96] is the stride-2 D-conv band matrix with edge
     replication folded into its entries (the D conv rides the contraction
     for free). H edge padding = 2+2 replicated SBUF rows, filled on-chip.
     fp32r matmuls must write PSUM base partition 0, so the two planes of a
     pair accumulate in separate PSUM tiles; PSUM->SBUF copies pack them
     into the partition halves of Y1 [128, 48(h'), 100(w pad)].

  S2 (shift-accumulate W), split across engines per pair to balance load:
     - PE pairs: 5 PSUM-accumulated matmuls per h'-block with scaled-identity
       lhsT (f1[kw] * I_128); both planes ride one K=128 contraction.
     - DVE pairs: tensor_scalar_mul + 4x scalar_tensor_tensor FMAs over
       stride-2 slices of Y1, accumulating in SBUF (no PSUM round trip).
     W edge pad = replicated Y1 columns, filled on-chip.

  Matmuls run as float32r (fast fp32 path, 1 cyc/row at N>=256).
  PSUM->SBUF copies alternate VectorE / ScalarE.
"""

import numpy as np

import concourse.bacc as bacc
import concourse.mybir as mybir
import concourse.tile as tile
from concourse.bass_utils import run_bass_kernel_spmd

F32 = mybir.dt.float32
F32R = mybir.dt.float32r

N_CORES = 8
PLANES = 8          # (b,c) planes per core: 2 batches x 4 channels
D = H = W = 96
DO = HO = WO = 48
HP = H + 4          # h-padded length in SBUF
WP = W + 4          # w-padded length of Y1
MP = 64             # padded d' rows per plane in PSUM partitions

F1 = (np.array([1.0, 4.0, 6.0, 4.0, 1.0]) / 16.0).astype(np.float32)

# h'-blocks: each S1 block spans two PSUM banks (two 5-row matmul sub-chains),
# one PSUM->SBUF copy per block.
S1_BLOCKS = [(0, 10), (10, 10), (20, 10), (30, 10), (40, 8)]
S2_BLOCKS = [(0, 20), (20, 20), (40, 8)]
# Engine running S2 for each plane-pair ("pe" or "dve").
S2_ENGINE = ("dve", "dve", "dve", "dve")
COPY_PATTERN = ("act",)  # all PSUM copies on ACT: DVE carries S2

_NC_CACHE = {}


def _band_matrix(f1: np.ndarray) -> np.ndarray:
    """Stride-2 conv band matrix [48, 96] with edge replication folded in."""
    a = np.zeros((DO, D), np.float32)
    for dp in range(DO):
        for k in range(5):
            d = min(max(2 * dp + k - 2, 0), D - 1)
            a[dp, d] += f1[k]
    return a


def build_weights(f1: np.ndarray):
    a_t = _band_matrix(f1).T  # [96, 48]
    wd = np.zeros((5, D, MP), np.float32)
    for kh in range(5):
        wd[kh, :, :DO] = f1[kh] * a_t
    eye = np.eye(128, dtype=np.float32)
    wi = np.zeros((5, 128, 128), np.float32)
    for kw in range(5):
        wi[kw] = f1[kw] * eye
    return wd, wi


def build_nc(reps: int = 1):
    nc = bacc.Bacc("TRN2", target_bir_lowering=False, debug=False)
    x = nc.dram_tensor("x", [PLANES, D, H, W], F32R, kind="ExternalInput")
    wd = nc.dram_tensor("wd", [5, D, MP], F32R, kind="ExternalInput")
    wi = nc.dram_tensor("wi", [5, 128, 128], F32R, kind="ExternalInput")
    y = nc.dram_tensor("y", [PLANES, DO, HO, WO], F32, kind="ExternalOutput")
    xap, yap = x[:], y[:]

    mult = mybir.AluOpType.mult
    add = mybir.AluOpType.add

    with tile.TileContext(nc) as tc:
        with (
            tc.tile_pool(name="wpool", bufs=1) as wpool,
            tc.tile_pool(name="xpool", bufs=3) as xpool,
            tc.tile_pool(name="y1pool", bufs=2) as y1pool,
            tc.tile_pool(name="opool", bufs=2) as opool,
            tc.tile_pool(name="ps1pool", bufs=4, space="PSUM") as ps1pool,
        ):
            wd_sb = wpool.tile([D, 5, MP], F32R)
            nc.sync.dma_start(wd_sb[:], wd[:].rearrange("k p m -> p k m"))
            wi_sb = wpool.tile([128, 5, 128], F32R)
            nc.sync.dma_start(wi_sb[:], wi[:].rearrange("k p m -> p k m"))

            ncopy = 0

            for rep in range(reps):
              for pair in range(PLANES // 2):
                planes = (2 * pair, 2 * pair + 1)
                s2_dve = S2_ENGINE[pair] == "dve"

                xts = []
                for p in planes:
                    xt = xpool.tile([D, HP, W], F32R, tag="xt", name=f"xt{rep}_{p}")
                    nc.sync.dma_start(xt[:, 2:54, :], xap[p, :, 0:52, :])
                    nc.sync.dma_start(xt[:, 54 : H + 2, :], xap[p, :, 52:H, :])
                    # replicated edge rows (h = -2, -1, 96, 97), filled on-chip
                    nc.gpsimd.tensor_copy(
                        xt[:, 0:2, :], xt[:, 2:3, :].broadcast_to((D, 2, W))
                    )
                    nc.gpsimd.tensor_copy(
                        xt[:, H + 2 : H + 4, :],
                        xt[:, H + 1 : H + 2, :].broadcast_to((D, 2, W)),
                    )
                    xts.append(xt)

                y1 = y1pool.tile([128, HO, WP], F32R, tag="y1", name=f"y1_{rep}_{pair}")

                # ---- S1: contract D, shift-accumulate H ----
                # PSUM tile = 2 banks [MP, 2, 512]; each bank holds one 5-row
                # matmul sub-chain (matmul output cannot cross a bank).
                for h0, hb in S1_BLOCKS:
                    for i in (0, 1):
                        ps1 = ps1pool.tile(
                            [MP, 2, 512], F32, tag="ps1", name=f"ps1_{rep}_{pair}_{h0}_{i}"
                        )
                        subs = []
                        for bank, s0 in enumerate(range(0, hb, 5)):
                            sb = min(5, hb - s0)
                            subs.append((bank, s0, sb))
                            out_ap = ps1[:, bank, : sb * W].rearrange(
                                "p (h w) -> p h w", h=sb
                            )
                            for kh in range(5):
                                hs = 2 * (h0 + s0) + kh
                                rhs = xts[i][:, hs : hs + 2 * sb - 1 : 2, :]
                                nc.tensor.matmul(
                                    out_ap,
                                    wd_sb[:, kh, :],
                                    rhs,
                                    start=(kh == 0),
                                    stop=(kh == 4),
                                )
                        if len(subs) == 2 and subs[0][2] == subs[1][2]:
                            # uniform banks: one copy covering both
                            sb = subs[0][2]
                            src = ps1[:, :, : sb * W].rearrange(
                                "p a (h w) -> p a h w", h=sb
                            )
                            dst4 = y1[
                                i * MP : (i + 1) * MP, h0 : h0 + hb, 2 : W + 2
                            ].rearrange("p (a h) w -> p a h w", a=2)
                            dsts = [dst4]
                            srcs = [src]
                        else:
                            dsts, srcs = [], []
                            for bank, s0, sb in subs:
                                dsts.append(
                                    y1[
                                        i * MP : (i + 1) * MP,
                                        h0 + s0 : h0 + s0 + sb,
                                        2 : W + 2,
                                    ]
                                )
                                srcs.append(
                                    ps1[:, bank, : sb * W].rearrange(
                                        "p (h w) -> p h w", h=sb
                                    )
                                )
                        for dst, src in zip(dsts, srcs):
                            if COPY_PATTERN[ncopy % len(COPY_PATTERN)] == "dve":
                                nc.vector.tensor_copy(dst, src)
                            else:
                                nc.scalar.copy(dst, src)
                            ncopy += 1

                # replicated edge columns of Y1 (w = -2, -1, 96, 97)
                for dst_c, src_c in ((0, 2), (1, 2), (W + 2, W + 1), (W + 3, W + 1)):
                    nc.scalar.copy(
                        y1[:, :, dst_c : dst_c + 1], y1[:, :, src_c : src_c + 1]
                    )
                    ncopy += 1

                # ---- S2: shift-accumulate W ----
                outs = opool.tile([128, HO, WO], F32, tag="outs", name=f"outs_{rep}_{pair}")
                if not s2_dve:
                    raise NotImplementedError("pe S2 disabled in this build")
                if s2_dve:
                    nc.vector.tensor_scalar_mul(
                        outs[:], y1[:, :, 0 : 2 * WO - 1 : 2], float(F1[0])
                    )
                    for kw in range(1, 5):
                        nc.vector.scalar_tensor_tensor(
                            outs[:],
                            y1[:, :, kw : kw + 2 * WO - 1 : 2],
                            float(F1[kw]),
                            outs[:],
                            op0=mult,
                            op1=add,
                        )
                else:
                    for h0, hb in S2_BLOCKS:
                        ps2 = ps2pool.tile(
                            [128, 2, 512], F32, tag="ps2", name=f"ps2_{rep}_{pair}_{h0}"
                        )
                        nsub = (hb + 9) // 10
                        for bank, s0 in enumerate(range(0, hb, 10)):
                            sb = min(10, hb - s0)
                            out_ap = ps2[:, bank, : sb * WO].rearrange(
                                "p (h w) -> p h w", h=sb
                            )
                            for kw in range(5):
                                rhs = y1[
                                    :, h0 + s0 : h0 + s0 + sb, kw : kw + 2 * WO - 1 : 2
                                ]
                                nc.tensor.matmul(
                                    out_ap,
                                    wi_sb[:, kw, :],
                                    rhs,
                                    start=(kw == 0),
                                    stop=(kw == 4),
                                )
                        if nsub == 2:
                            src = ps2[:, :, : 10 * WO].rearrange(
                                "p a (h w) -> p a h w", h=10
                            )
                            dst = outs[:, h0 : h0 + hb, :].rearrange(
                                "p (a h) w -> p a h w", a=2
                            )
                        else:
                            src = ps2[:, 0, : hb * WO].rearrange(
                                "p (h w) -> p h w", h=hb
                            )
                            dst = outs[:, h0 : h0 + hb, :]
                        if ncopy % 2 == 0:
                            nc.vector.tensor_copy(dst, src)
                        else:
                            nc.scalar.copy(dst, src)
                        ncopy += 1

                # stores ride the ACT HWDGE queue so they cannot head-of-line
                # block the SP queue that feeds the next pair's loads
                nc.scalar.dma_start(yap[planes[0]], outs[0:DO])
                nc.scalar.dma_start(yap[planes[1]], outs[MP : MP + DO])

    nc.compile()
    return nc


def _get_nc():
    if "nc" not in _NC_CACHE:
        _NC_CACHE["nc"] = build_nc()
    return _NC_CACHE["nc"]


def kernel(x: np.ndarray, f: np.ndarray | None = None, **_unused) -> np.ndarray:
    x = np.ascontiguousarray(np.asarray(x, dtype=np.float32))
    B, CH = x.shape[0], x.shape[1]
    if f is not None:
        f3 = np.asarray(f, dtype=np.float64)[0, 0]
        f1 = f3.sum(axis=(1, 2))
        f1 = (f1 / f1.sum()).astype(np.float32)
    else:
        f1 = F1

    wd, wi = build_weights(f1)
    nc = _get_nc()

    ch_per_core = CH // N_CORES
    in_maps = []
    for c in range(N_CORES):
        shard = x[:, c * ch_per_core : (c + 1) * ch_per_core]
        in_maps.append(
            {
                "x": np.ascontiguousarray(shard.reshape(PLANES, D, H, W)),
                "wd": wd,
                "wi": wi,
            }
        )

    res = run_bass_kernel_spmd(nc, in_maps, core_ids=list(range(N_CORES)))
    outs = [r["y"].reshape(B, ch_per_core, DO, HO, WO) for r in res.results]
    return np.concatenate(outs, axis=1)


if __name__ == "__main__":
    rng = np.random.default_rng(0)
    x = rng.standard_normal((2, 32, D, H, W), dtype=np.float32)
    out = kernel(x)
    print(out.shape, out.dtype)


# revision 15
# speedup vs baseline: 1.2235x; 1.2235x over previous
"""BlurPool3D Trainium2 kernel.

Depthwise 5x5x5 binomial blur, stride 2, edge padding, on x [2, 32, 96, 96, 96] f32.
Output [2, 32, 48, 48, 48] f32.

Strategy (per NeuronCore, channels sharded 8-ways -> 8 (b,c) planes/core):
  The filter is separable: f = f1 x f1 x f1, f1 = [1,4,6,4,1]/16.
  Per plane [D, H, W] = [96, 96, 96], no transposes anywhere:

  S1 (TensorE: contract D + shift-accumulate H): SBUF X [96(d), 100(h pad), 96(w)].
     For each h'-block, 5 PSUM-accumulated matmuls (one per H tap kh):
       psum[d', (h',w)] += (f1[kh] * A_D^T).T @ X[:, 2h'+kh, :]
     where A_D [48,96] is the stride-2 D-conv band matrix with edge
     replication folded into its entries (the D conv rides the contraction
     for free). H edge padding = 2+2 replicated SBUF rows, filled on-chip.
     fp32r matmuls must write PSUM base partition 0, so the two planes of a
     pair accumulate in separate PSUM tiles; PSUM->SBUF copies pack them
     into the partition halves of Y1 [128, 48(h'), 100(w pad)].

  S2 (shift-accumulate W), split across engines per pair to balance load:
     - PE pairs: 5 PSUM-accumulated matmuls per h'-block with scaled-identity
       lhsT (f1[kw] * I_128); both planes ride one K=128 contraction.
     - DVE pairs: tensor_scalar_mul + 4x scalar_tensor_tensor FMAs over
       stride-2 slices of Y1, accumulating in SBUF (no PSUM round trip).
     W edge pad = replicated Y1 columns, filled on-chip.

  Matmuls run as float32r (fast fp32 path, 1 cyc/row at N>=256).
  PSUM->SBUF copies alternate VectorE / ScalarE.
"""

import numpy as np

import concourse.bacc as bacc
import concourse.mybir as mybir
import concourse.tile as tile
from concourse.bass_utils import run_bass_kernel_spmd

F32 = mybir.dt.float32
F32R = mybir.dt.float32r

N_CORES = 8
PLANES = 8          # (b,c) planes per core: 2 batches x 4 channels
D = H = W = 96
DO = HO = WO = 48
HP = H + 4          # h-padded length in SBUF
WP = W + 4          # w-padded length of Y1
MP = 64             # padded d' rows per plane in PSUM partitions

F1 = (np.array([1.0, 4.0, 6.0, 4.0, 1.0]) / 16.0).astype(np.float32)

# h'-blocks: each S1 block spans two PSUM banks (two 5-row matmul sub-chains),
# one PSUM->SBUF copy per block.
S1_BLOCKS = [(0, 10), (10, 10), (20, 10), (30, 10), (40, 8)]
S2_BLOCKS = [(0, 20), (20, 20), (40, 8)]
# Engine running S2 for each plane-pair ("pe" or "dve").
S2_ENGINE = ("dve", "dve", "dve", "dve")
COPY_PATTERN = ("act", "act", "dve")  # 2/3 on ACT: DVE carries S2

_NC_CACHE = {}


def _band_matrix(f1: np.ndarray) -> np.ndarray:
    """Stride-2 conv band matrix [48, 96] with edge replication folded in."""
    a = np.zeros((DO, D), np.float32)
    for dp in range(DO):
        for k in range(5):
            d = min(max(2 * dp + k - 2, 0), D - 1)
            a[dp, d] += f1[k]
    return a


def build_weights(f1: np.ndarray):
    a_t = _band_matrix(f1).T  # [96, 48]
    wd = np.zeros((5, D, MP), np.float32)
    for kh in range(5):
        wd[kh, :, :DO] = f1[kh] * a_t
    eye = np.eye(128, dtype=np.float32)
    wi = np.zeros((5, 128, 128), np.float32)
    for kw in range(5):
        wi[kw] = f1[kw] * eye
    return wd, wi


def build_nc(reps: int = 1):
    nc = bacc.Bacc("TRN2", target_bir_lowering=False, debug=False)
    x = nc.dram_tensor("x", [PLANES, D, H, W], F32R, kind="ExternalInput")
    wd = nc.dram_tensor("wd", [5, D, MP], F32R, kind="ExternalInput")
    wi = nc.dram_tensor("wi", [5, 128, 128], F32R, kind="ExternalInput")
    y = nc.dram_tensor("y", [PLANES, DO, HO, WO], F32, kind="ExternalOutput")
    xap, yap = x[:], y[:]

    mult = mybir.AluOpType.mult
    add = mybir.AluOpType.add

    with tile.TileContext(nc) as tc:
        with (
            tc.tile_pool(name="wpool", bufs=1) as wpool,
            tc.tile_pool(name="xpool", bufs=3) as xpool,
            tc.tile_pool(name="y1pool", bufs=2) as y1pool,
            tc.tile_pool(name="opool", bufs=2) as opool,
            tc.tile_pool(name="ps1pool", bufs=4, space="PSUM") as ps1pool,
        ):
            wd_sb = wpool.tile([D, 5, MP], F32R)
            nc.sync.dma_start(wd_sb[:], wd[:].rearrange("k p m -> p k m"))
            wi_sb = wpool.tile([128, 5, 128], F32R)
            nc.sync.dma_start(wi_sb[:], wi[:].rearrange("k p m -> p k m"))

            ncopy = 0

            for rep in range(reps):
              for pair in range(PLANES // 2):
                planes = (2 * pair, 2 * pair + 1)
                s2_dve = S2_ENGINE[pair] == "dve"

                xts = []
                for p in planes:
                    xt = xpool.tile([D, HP, W], F32R, tag="xt", name=f"xt{rep}_{p}")
                    for c0, c1 in ((0, 28), (28, 52), (52, 76), (76, H)):
                        nc.sync.dma_start(
                            xt[:, c0 + 2 : c1 + 2, :], xap[p, :, c0:c1, :]
                        )
                    # replicated edge rows (h = -2, -1, 96, 97), filled on-chip
                    nc.gpsimd.tensor_copy(
                        xt[:, 0:2, :], xt[:, 2:3, :].broadcast_to((D, 2, W))
                    )
                    nc.gpsimd.tensor_copy(
                        xt[:, H + 2 : H + 4, :],
                        xt[:, H + 1 : H + 2, :].broadcast_to((D, 2, W)),
                    )
                    xts.append(xt)

                y1 = y1pool.tile([128, HO, WP], F32R, tag="y1", name=f"y1_{rep}_{pair}")

                # ---- S1: contract D, shift-accumulate H ----
                # PSUM tile = 2 banks [MP, 2, 512]; each bank holds one 5-row
                # matmul sub-chain (matmul output cannot cross a bank).
                for h0, hb in S1_BLOCKS:
                    for i in (0, 1):
                        ps1 = ps1pool.tile(
                            [MP, 2, 512], F32, tag="ps1", name=f"ps1_{rep}_{pair}_{h0}_{i}"
                        )
                        subs = []
                        for bank, s0 in enumerate(range(0, hb, 5)):
                            sb = min(5, hb - s0)
                            subs.append((bank, s0, sb))
                            out_ap = ps1[:, bank, : sb * W].rearrange(
                                "p (h w) -> p h w", h=sb
                            )
                            for kh in range(5):
                                hs = 2 * (h0 + s0) + kh
                                rhs = xts[i][:, hs : hs + 2 * sb - 1 : 2, :]
                                nc.tensor.matmul(
                                    out_ap,
                                    wd_sb[:, kh, :],
                                    rhs,
                                    start=(kh == 0),
                                    stop=(kh == 4),
                                )
                        if len(subs) == 2 and subs[0][2] == subs[1][2]:
                            # uniform banks: one copy covering both
                            sb = subs[0][2]
                            src = ps1[:, :, : sb * W].rearrange(
                                "p a (h w) -> p a h w", h=sb
                            )
                            dst4 = y1[
                                i * MP : (i + 1) * MP, h0 : h0 + hb, 2 : W + 2
                            ].rearrange("p (a h) w -> p a h w", a=2)
                            dsts = [dst4]
                            srcs = [src]
                        else:
                            dsts, srcs = [], []
                            for bank, s0, sb in subs:
                                dsts.append(
                                    y1[
                                        i * MP : (i + 1) * MP,
                                        h0 + s0 : h0 + s0 + sb,
                                        2 : W + 2,
                                    ]
                                )
                                srcs.append(
                                    ps1[:, bank, : sb * W].rearrange(
                                        "p (h w) -> p h w", h=sb
                                    )
                                )
                        for dst, src in zip(dsts, srcs):
                            if COPY_PATTERN[ncopy % len(COPY_PATTERN)] == "dve":
                                nc.vector.tensor_copy(dst, src)
                            else:
                                nc.scalar.copy(dst, src)
                            ncopy += 1

                # replicated edge columns of Y1 (w = -2, -1, 96, 97)
                nc.gpsimd.tensor_copy(
                    y1[:, :, 0:2], y1[:, :, 2:3].broadcast_to((128, HO, 2))
                )
                nc.gpsimd.tensor_copy(
                    y1[:, :, W + 2 : W + 4],
                    y1[:, :, W + 1 : W + 2].broadcast_to((128, HO, 2)),
                )

                # ---- S2: shift-accumulate W ----
                outs = opool.tile([128, HO, WO], F32, tag="outs", name=f"outs_{rep}_{pair}")
                if not s2_dve:
                    raise NotImplementedError("pe S2 disabled in this build")
                if s2_dve:
                    for g0, g1 in ((0, 24), (24, HO)):
                        nc.vector.tensor_scalar_mul(
                            outs[:, g0:g1, :],
                            y1[:, g0:g1, 0 : 2 * WO - 1 : 2],
                            float(F1[0]),
                        )
                        for kw in range(1, 5):
                            nc.vector.scalar_tensor_tensor(
                                outs[:, g0:g1, :],
                                y1[:, g0:g1, kw : kw + 2 * WO - 1 : 2],
                                float(F1[kw]),
                                outs[:, g0:g1, :],
                                op0=mult,
                                op1=add,
                            )
                else:
                    for h0, hb in S2_BLOCKS:
                        ps2 = ps2pool.tile(
                            [128, 2, 512], F32, tag="ps2", name=f"ps2_{rep}_{pair}_{h0}"
                        )
                        nsub = (hb + 9) // 10
                        for bank, s0 in enumerate(range(0, hb, 10)):
                            sb = min(10, hb - s0)
                            out_ap = ps2[:, bank, : sb * WO].rearrange(
                                "p (h w) -> p h w", h=sb
                            )
                            for kw in range(5):
                                rhs = y1[
                                    :, h0 + s0 : h0 + s0 + sb, kw : kw + 2 * WO - 1 : 2
                                ]
                                nc.tensor.matmul(
                                    out_ap,
                                    wi_sb[:, kw, :],
                                    rhs,
                                    start=(kw == 0),
                                    stop=(kw == 4),
                                )
                        if nsub == 2:
                            src = ps2[:, :, : 10 * WO].rearrange(
                                "p a (h w) -> p a h w", h=10
                            )
                            dst = outs[:, h0 : h0 + hb, :].rearrange(
                                "p (a h) w -> p a h w", a=2
                            )
                        else:
                            src = ps2[:, 0, : hb * WO].rearrange(
                                "p (h w) -> p h w", h=hb
                            )
                            dst = outs[:, h0 : h0 + hb, :]
                        if ncopy % 2 == 0:
                            nc.vector.tensor_copy(dst, src)
                        else:
                            nc.scalar.copy(dst, src)
                        ncopy += 1

                # stores ride the ACT HWDGE queue so they cannot head-of-line
                # block the SP queue that feeds the next pair's loads
                nc.scalar.dma_start(yap[planes[0]], outs[0:DO])
                nc.scalar.dma_start(yap[planes[1]], outs[MP : MP + DO])

    nc.compile()
    return nc


def _get_nc():
    if "nc" not in _NC_CACHE:
        _NC_CACHE["nc"] = build_nc()
    return _NC_CACHE["nc"]


def kernel(x: np.ndarray, f: np.ndarray | None = None, **_unused) -> np.ndarray:
    x = np.ascontiguousarray(np.asarray(x, dtype=np.float32))
    B, CH = x.shape[0], x.shape[1]
    if f is not None:
        f3 = np.asarray(f, dtype=np.float64)[0, 0]
        f1 = f3.sum(axis=(1, 2))
        f1 = (f1 / f1.sum()).astype(np.float32)
    else:
        f1 = F1

    wd, wi = build_weights(f1)
    nc = _get_nc()

    ch_per_core = CH // N_CORES
    in_maps = []
    for c in range(N_CORES):
        shard = x[:, c * ch_per_core : (c + 1) * ch_per_core]
        in_maps.append(
            {
                "x": np.ascontiguousarray(shard.reshape(PLANES, D, H, W)),
                "wd": wd,
                "wi": wi,
            }
        )

    res = run_bass_kernel_spmd(nc, in_maps, core_ids=list(range(N_CORES)))
    outs = [r["y"].reshape(B, ch_per_core, DO, HO, WO) for r in res.results]
    return np.concatenate(outs, axis=1)


if __name__ == "__main__":
    rng = np.random.default_rng(0)
    x = rng.standard_normal((2, 32, D, H, W), dtype=np.float32)
    out = kernel(x)
    print(out.shape, out.dtype)
